# revision 38
# baseline (speedup 1.0000x reference)
"""Trainium2 Bass kernel for nn_Causal_model_vae (MoE-routed VAE).

Reference computation (N=16384 tokens, DX=DH=1024, S=8 experts):
    mu_h     = leaky(data @ Wm1 + bm1) @ Wm2 + bm2
    logvar_h = leaky(data @ Wv1 + bv1) @ Wv2 + bv2
    h_sample = eps * exp(0.5*logvar_h) + mu_h
    reconstruct[n] = (leaky(h_sample @ We1[s_n] + be1[s_n]) @ We2[s_n] + be2[s_n])
returns (reconstruct, mu_h, logvar_h, h_sample).

Strategy: expert-parallel across the 8 NeuronCores.  The routing ids `s` are
known on the host, so the host sorts tokens by expert, pads each expert's
token list to a common capacity C (= max expert count rounded up to 128),
and core e processes exactly expert e's tokens: the (replicated-weight)
encoder on its C tokens, then ONLY its own expert's decoder — 6 matmul
layers per token instead of the reference's dense 4 + 2*S.

On-chip layout: everything is feature-major [feature, token] so chained
matmuls need no transposes; matmul operands are bf16 (f32 PSUM accumulation),
the sampling chain and all outputs are f32.  All DRAM tensors use the flat
SBUF-image layout [128, KT*width] (host pre/post-arranges) so every DMA is
fully contiguous per partition — strided DMAs cost ~2-3.5us of descriptor
generation on the sync sequencer and stall the pipeline.

The decoder is software-pipelined one token-block behind the encoder so the
PE never idles while the sampling chain (ACT exp + DVE fma) drains.

Biases are structurally zero in this problem's setup_inputs(); the kernel
asserts that and skips them on-device.
"""

import contextlib
import ctypes
import math
import os
import sys
import types

import numpy as np
import ml_dtypes

import concourse.bacc as bacc
import concourse.bass as bass
import concourse.mybir as mybir
import concourse.tile as tile
from concourse.bass_utils import run_bass_kernel_spmd

N, DX, DH, S = 16384, 1024, 1024, 8
KT = DH // 128    # 8 k-tiles (DX == DH == 1024)
SLOPE = 0.01
NCORES = 8
T = 256           # main token block width (matmul moving dim)
C_MIN = 256       # capacity floor; C = ceil(max expert count / 128) * 128

BF16 = mybir.dt.bfloat16
F32 = mybir.dt.float32

LAST_RESULTS = None  # BassKernelResults of the most recent run (for profiling)

_program_cache: dict[int, "bacc.Bacc"] = {}


def _ensure_ntff_hook():
    """bass_utils imports antenv.axon_hooks when tracing under axon; some
    images lack that module.  Install a ctypes-based equivalent if so."""
    try:
        import antenv.axon_hooks  # noqa: F401
        return
    except ImportError:
        pass
    try:
        import antenv

        so_path = "/opt/axon/libaxon_pjrt.so"
        if not os.path.exists(so_path):
            return
        lib = ctypes.CDLL(so_path)
        if not hasattr(lib, "axon_start_nrt_profile"):
            return
        lib.axon_start_nrt_profile.argtypes = [
            ctypes.POINTER(ctypes.c_int64), ctypes.c_size_t]
        lib.axon_start_nrt_profile.restype = ctypes.c_int64
        lib.axon_stop_nrt_profile.argtypes = [ctypes.c_char_p]
        lib.axon_stop_nrt_profile.restype = ctypes.c_int64

        @contextlib.contextmanager
        def _hook(output_dir, device_ids):
            import jax

            jax.devices()
            if device_ids:
                ids = (ctypes.c_int64 * len(device_ids))(*device_ids)
                rc = lib.axon_start_nrt_profile(ids, len(device_ids))
            else:
                rc = lib.axon_start_nrt_profile(None, 0)
            if rc != 0:
                raise RuntimeError(f"axon_start_nrt_profile rc={rc}")
            try:
                yield
            finally:
                n = lib.axon_stop_nrt_profile(str(output_dir).encode())
                print(f"ntff profile: {n} file(s) -> {output_dir}")

        m = types.ModuleType("antenv.axon_hooks")
        m.get_axon_ntff_profile_hook = lambda: _hook
        m.set_axon_ntff_profile_hook = lambda h: None
        sys.modules["antenv.axon_hooks"] = m
        antenv.axon_hooks = m
    except Exception:
        pass


def _token_blocks(C):
    blocks = [(i * T, T) for i in range(C // T)]
    if C % T:
        blocks.append((C - C % T, C % T))
    return blocks


def build_program(C: int) -> "bacc.Bacc":
    assert C % 128 == 0
    blocks = _token_blocks(C)
    nblocks = len(blocks)

    nc = bacc.Bacc("TRN2", target_bir_lowering=False, debug=False,
                   num_devices=NCORES)

    xT = nc.dram_tensor("xT", [128, KT * C], BF16, kind="ExternalInput").ap()
    epsT = nc.dram_tensor("epsT", [128, KT * C], F32, kind="ExternalInput").ap()
    wnames = ["wm1", "wv1", "wm2", "wv2", "we1", "we2"]
    wdram = {n: nc.dram_tensor(n, [128, KT * 1024], BF16,
                               kind="ExternalInput").ap() for n in wnames}
    outs = {n: nc.dram_tensor(n, [128, KT * C], F32,
                              kind="ExternalOutput").ap()
            for n in ["muT", "lvT", "hT", "recT"]}

    Exp = mybir.ActivationFunctionType.Exp
    Copy = mybir.ActivationFunctionType.Copy
    mult = mybir.AluOpType.mult
    max_ = mybir.AluOpType.max
    add = mybir.AluOpType.add

    with tile.TileContext(nc) as tc:
        with (
            tc.tile_pool(name="wpool", bufs=1) as wpool,
            tc.tile_pool(name="io2", bufs=2) as io2,
            tc.tile_pool(name="io", bufs=1) as io,
            tc.tile_pool(name="mid", bufs=1) as mid,
            # Separate PSUM pools so the (one-block-delayed) decoder's
            # slot-recycling waits never reference encoder matmul progress.
            tc.tile_pool(name="psum_e", bufs=6,
                         space=bass.MemorySpace.PSUM) as psum_e,
            tc.tile_pool(name="psum_d", bufs=2,
                         space=bass.MemorySpace.PSUM) as psum_d,
        ):
            # Block-0 inputs first so their DMA descriptors lead the queues,
            # then resident weights in usage order: the first matmuls wait
            # only on x.k0 + wm1.k0, eps comes when the sampling needs it,
            # and the expert weights (first needed after enc(1)) come last.
            xt_tiles = {}
            eps_tiles = {}

            def fetch_block(b, x_only=False, eng=None):
                if b >= nblocks:
                    return
                eng = eng or nc.sync
                off, w = blocks[b]
                if b not in xt_tiles:
                    x = io2.tile([128, KT * w], BF16, tag="x")
                    eng.dma_start(x[:], xT[:, off * KT : off * KT + KT * w])
                    xt_tiles[b] = x
                if not x_only and b not in eps_tiles:
                    e = io2.tile([128, KT * w], F32, tag="eps")
                    eng.dma_start(e[:], epsT[:, off * KT : off * KT + KT * w])
                    eps_tiles[b] = e

            wt = {}

            def fetch_weight(name, eng=None):
                eng = eng or nc.sync
                w = wpool.tile([128, KT * 1024], BF16, tag=f"w_{name}")
                eng.dma_start(w[:], wdram[name][:])
                wt[name] = w

            # Head: the first matmuls' dependencies (x0, wm1, wv1, ...) get a
            # short private sync queue; everything not needed until later
            # desc-gens concurrently on the scalar engine's HWDGE queue.
            fetch_block(0, x_only=True)
            for n in ["wm1", "wv1", "wm2", "wv2"]:
                fetch_weight(n)
            fetch_block(0, eng=nc.scalar)
            fetch_block(1, eng=nc.scalar)
            for n in ["we1", "we2"]:
                fetch_weight(n, eng=nc.scalar)

            def layer(w, rhs, tw, out_cb, pool=None, first_after=None):
                """One 1024->1024 matmul layer on a [128, KT*tw] bf16 rhs.

                out_cb(mp, ps) consumes the [128, 2*tw] f32 psum of m-pair mp.
                Returns the last matmul instruction.  first_after: ordering
                hint — schedule this layer's first matmul after that inst.
                """
                pool = pool or psum_e
                mm = None
                for mp in range(4):
                    ps = pool.tile([128, 2 * tw], F32, tag="ps")
                    for half in range(2):
                        m = 2 * mp + half
                        for k in range(KT):
                            mm = nc.tensor.matmul(
                                ps[:, half * tw : (half + 1) * tw],
                                w[:, k * 1024 + m * 128 : k * 1024 + (m + 1) * 128],
                                rhs[:, k * tw : (k + 1) * tw],
                                start=(k == 0),
                                stop=(k == KT - 1),
                            )
                            if first_after is not None:
                                tile.add_dep_helper(
                                    mm.ins, first_after.ins, sync=False,
                                    reason="decoder pipelined behind next block")
                                first_after = None
                    out_cb(mp, ps)
                return mm

            def leaky_to(dst, tw):
                def cb(mp, ps):
                    # leaky(x) = max(x, 0.01x); DVE can read PSUM only
                    # once per op, so stage 0.01x in SBUF first.
                    lk = io2.tile([128, 2 * tw], F32, tag="lk")
                    nc.vector.tensor_scalar_mul(lk[:], ps[:], SLOPE)
                    nc.vector.tensor_tensor(
                        dst[:, 2 * mp * tw : (2 * mp + 2) * tw],
                        lk[:], ps[:], max_)
                return cb

            def enc_block(b):
                """Encoder + sampling for block b; returns the bf16 h tile."""
                off, tw = blocks[b]
                x, epst = xt_tiles.pop(b), eps_tiles.pop(b)
                fetch_block(b + 1)

                h1m = mid.tile([128, KT * tw], BF16, tag="h1m")
                l1m_last = layer(wt["wm1"], x, tw, leaky_to(h1m, tw))
                h1v = mid.tile([128, KT * tw], BF16, tag="h1v")
                layer(wt["wv1"], x, tw, leaky_to(h1v, tw))

                mu_f = io.tile([128, KT * tw], F32, tag="mu_f")

                def mu_cb(mp, ps):
                    nc.scalar.activation(
                        mu_f[:, 2 * mp * tw : (2 * mp + 2) * tw], ps[:], Copy)

                layer(wt["wm2"], h1m, tw, mu_cb)
                nc.sync.dma_start(outs["muT"][:, off * KT : off * KT + KT * tw],
                                  mu_f[:])

                lv_f = io.tile([128, KT * tw], F32, tag="lv_f")
                std_f = mid.tile([128, KT * tw], F32, tag="std_f")
                tmp_f = mid.tile([128, KT * tw], F32, tag="tmp_f")
                h_f = io.tile([128, KT * tw], F32, tag="h_f")
                h_b = io2.tile([128, KT * tw], BF16, tag="h_b")

                def lv_cb(mp, ps):
                    sl = slice(2 * mp * tw, (2 * mp + 2) * tw)
                    nc.scalar.activation(lv_f[:, sl], ps[:], Copy)
                    nc.scalar.activation(std_f[:, sl], ps[:], Exp, scale=0.5)
                    # h = eps*std + mu, per m-pair so it pipelines
                    nc.vector.tensor_tensor(
                        tmp_f[:, sl], epst[:, sl], std_f[:, sl], mult)
                    nc.vector.tensor_tensor(
                        h_f[:, sl], tmp_f[:, sl], mu_f[:, sl], add)
                    nc.vector.tensor_tensor(
                        h_b[:, sl], tmp_f[:, sl], mu_f[:, sl], add)

                layer(wt["wv2"], h1v, tw, lv_cb)
                nc.sync.dma_start(outs["lvT"][:, off * KT : off * KT + KT * tw],
                                  lv_f[:])
                nc.sync.dma_start(outs["hT"][:, off * KT : off * KT + KT * tw],
                                  h_f[:])
                return h_b, l1m_last

            def dec_block(b, h_b, after=None):
                """Decoder (this core's expert) for block b."""
                off, tw = blocks[b]
                d1 = mid.tile([128, KT * tw], BF16, tag="d1")
                layer(wt["we1"], h_b, tw, leaky_to(d1, tw), pool=psum_d,
                      first_after=after)

                rec_f = io.tile([128, KT * tw], F32, tag="rec_f")

                def rec_cb(mp, ps):
                    sl = slice(2 * mp * tw, (2 * mp + 2) * tw)
                    nc.scalar.activation(rec_f[:, sl], ps[:], Copy)
                    # per-m-pair output DMA so the tail drains early
                    nc.sync.dma_start(
                        outs["recT"][:, off * KT + 2 * mp * tw :
                                     off * KT + (2 * mp + 2) * tw],
                        rec_f[:, sl])

                layer(wt["we2"], d1, tw, rec_cb, pool=psum_d)

            # Software-pipeline the decoder one block behind the encoder:
            # while block b's sampling chain (ACT exp + DVE fma) drains,
            # the PE is busy on block b-1's decoder — no PE idle at block
            # boundaries (which would also re-throttle the HAM clock).
            prev = None
            for b in range(nblocks):
                h_b, l1m_last = enc_block(b)
                if prev is not None:
                    dec_block(b - 1, prev, after=l1m_last)
                prev = h_b
            dec_block(nblocks - 1, prev)

    nc.compile()
    return nc


def _get_program(C: int) -> "bacc.Bacc":
    if C not in _program_cache:
        _program_cache[C] = build_program(C)
    return _program_cache[C]


def _to_sbuf_image(arrT, blocks):
    """[1024, C] feature-major -> [128, KT*C] flat SBUF image, blockwise."""
    out = np.empty((128, KT * arrT.shape[1]), dtype=arrT.dtype)
    for off, w in blocks:
        seg = arrT[:, off:off + w].reshape(KT, 128, w).transpose(1, 0, 2)
        out[:, off * KT : off * KT + KT * w] = seg.reshape(128, KT * w)
    return out


def _from_sbuf_image(img, blocks, C):
    """[128, KT*C] flat SBUF image -> [1024, C] feature-major."""
    out = np.empty((1024, C), dtype=img.dtype)
    for off, w in blocks:
        seg = img[:, off * KT : off * KT + KT * w].reshape(128, KT, w)
        out[:, off:off + w] = seg.transpose(1, 0, 2).reshape(1024, w)
    return out


def _weight_image(W):
    """[1024 din, 1024 dout] -> [128, KT*1024] flat lhsT image."""
    return np.ascontiguousarray(
        W.reshape(KT, 128, 1024).transpose(1, 0, 2).reshape(128, KT * 1024))


def _kernel_numpy(inputs):
    """Exact f32 fallback (used only if an assumption is violated)."""
    d = {k: np.asarray(v) for k, v in inputs.items()}
    leaky = lambda v: np.where(v > 0, v, np.float32(SLOPE) * v)
    mu = leaky(d["data"] @ d["Wm1"] + d["bm1"]) @ d["Wm2"] + d["bm2"]
    lv = leaky(d["data"] @ d["Wv1"] + d["bv1"]) @ d["Wv2"] + d["bv2"]
    h = d["eps"] * np.exp(0.5 * lv) + mu
    s = np.asarray(d["s"]).astype(np.int64)
    rec = np.empty_like(d["data"])
    for e in range(d["We1"].shape[0]):
        m = s == e
        rec[m] = leaky(h[m] @ d["We1"][e] + d["be1"][e]) @ d["We2"][e] + d["be2"][e]
    return rec, mu, lv, h


def kernel(**inputs) -> tuple:
    data = np.ascontiguousarray(np.asarray(inputs["data"], dtype=np.float32))
    eps = np.ascontiguousarray(np.asarray(inputs["eps"], dtype=np.float32))
    s = np.asarray(inputs["s"]).astype(np.int64)
    # The device kernel folds the (structurally zero) biases away; any
    # violated assumption falls back to an exact host computation.
    nonzero_bias = any(
        np.abs(np.asarray(inputs[b])).max() != 0.0
        for b in ("bm1", "bm2", "bv1", "bv2", "be1", "be2"))
    if nonzero_bias or data.shape != (N, DX) or s.shape != (N,):
        return _kernel_numpy(inputs)

    counts = np.bincount(s, minlength=S)
    C = max(C_MIN, int(math.ceil(counts.max() / 128)) * 128)
    blocks = _token_blocks(C)
    nc = _get_program(C)

    bf = ml_dtypes.bfloat16
    # token ids per expert, padded to C with token 0 (results discarded)
    idx = np.zeros((S, C), dtype=np.int64)
    for e in range(S):
        ids = np.nonzero(s == e)[0]
        idx[e, : len(ids)] = ids

    wimg = {
        "wm1": _weight_image(np.asarray(inputs["Wm1"], np.float32)).astype(bf),
        "wm2": _weight_image(np.asarray(inputs["Wm2"], np.float32)).astype(bf),
        "wv1": _weight_image(np.asarray(inputs["Wv1"], np.float32)).astype(bf),
        "wv2": _weight_image(np.asarray(inputs["Wv2"], np.float32)).astype(bf),
    }
    We1 = np.asarray(inputs["We1"], np.float32)
    We2 = np.asarray(inputs["We2"], np.float32)
    dataT = data.T
    epsT = eps.T

    in_maps = []
    for e in range(S):
        ids = idx[e]
        in_maps.append({
            "xT": _to_sbuf_image(
                np.ascontiguousarray(dataT[:, ids]).astype(bf), blocks),
            "epsT": _to_sbuf_image(
                np.ascontiguousarray(epsT[:, ids]), blocks),
            "wm1": wimg["wm1"], "wm2": wimg["wm2"],
            "wv1": wimg["wv1"], "wv2": wimg["wv2"],
            "we1": _weight_image(We1[e]).astype(bf),
            "we2": _weight_image(We2[e]).astype(bf),
        })

    global LAST_RESULTS
    _ensure_ntff_hook()
    res = run_bass_kernel_spmd(nc, in_maps, list(range(NCORES)))
    LAST_RESULTS = res

    mu = np.empty((N, DH), np.float32)
    lv = np.empty((N, DH), np.float32)
    h = np.empty((N, DH), np.float32)
    rec = np.empty((N, DX), np.float32)
    for e in range(S):
        cnt = int(counts[e])
        ids = idx[e, :cnt]
        r = res.results[e]
        mu[ids] = _from_sbuf_image(r["muT"], blocks, C)[:, :cnt].T
        lv[ids] = _from_sbuf_image(r["lvT"], blocks, C)[:, :cnt].T
        h[ids] = _from_sbuf_image(r["hT"], blocks, C)[:, :cnt].T
        rec[ids] = _from_sbuf_image(r["recT"], blocks, C)[:, :cnt].T
    return rec, mu, lv, h


# revision 40
# speedup vs baseline: 1.0459x; 1.0459x over previous
"""Trainium2 Bass kernel for nn_Causal_model_vae (MoE-routed VAE).

Reference computation (N=16384 tokens, DX=DH=1024, S=8 experts):
    mu_h     = leaky(data @ Wm1 + bm1) @ Wm2 + bm2
    logvar_h = leaky(data @ Wv1 + bv1) @ Wv2 + bv2
    h_sample = eps * exp(0.5*logvar_h) + mu_h
    reconstruct[n] = (leaky(h_sample @ We1[s_n] + be1[s_n]) @ We2[s_n] + be2[s_n])
returns (reconstruct, mu_h, logvar_h, h_sample).

Strategy: expert-parallel across the 8 NeuronCores.  The routing ids `s` are
known on the host, so the host sorts tokens by expert, pads each expert's
token list to a common capacity C (= max expert count rounded up to 128),
and core e processes exactly expert e's tokens: the (replicated-weight)
encoder on its C tokens, then ONLY its own expert's decoder — 6 matmul
layers per token instead of the reference's dense 4 + 2*S.

On-chip layout: everything is feature-major [feature, token] so chained
matmuls need no transposes; matmul operands are bf16 (f32 PSUM accumulation),
the sampling chain and all outputs are f32.  All DRAM tensors use the flat
SBUF-image layout [128, KT*width] (host pre/post-arranges) so every DMA is
fully contiguous per partition — strided DMAs cost ~2-3.5us of descriptor
generation on the sync sequencer and stall the pipeline.

The decoder is software-pipelined one token-block behind the encoder so the
PE never idles while the sampling chain (ACT exp + DVE fma) drains.

Biases are structurally zero in this problem's setup_inputs(); the kernel
asserts that and skips them on-device.
"""

import contextlib
import ctypes
import math
import os
import sys
import types

import numpy as np
import ml_dtypes

import concourse.bacc as bacc
import concourse.bass as bass
import concourse.mybir as mybir
import concourse.tile as tile
from concourse.bass_utils import run_bass_kernel_spmd

N, DX, DH, S = 16384, 1024, 1024, 8
KT = DH // 128    # 8 k-tiles (DX == DH == 1024)
SLOPE = 0.01
NCORES = 8
T = 256           # main token block width (matmul moving dim)
C_MIN = 256       # capacity floor; C = ceil(max expert count / 128) * 128

BF16 = mybir.dt.bfloat16
F32 = mybir.dt.float32

LAST_RESULTS = None  # BassKernelResults of the most recent run (for profiling)

_program_cache: dict[int, "bacc.Bacc"] = {}


def _ensure_ntff_hook():
    """bass_utils imports antenv.axon_hooks when tracing under axon; some
    images lack that module.  Install a ctypes-based equivalent if so."""
    try:
        import antenv.axon_hooks  # noqa: F401
        return
    except ImportError:
        pass
    try:
        import antenv

        so_path = "/opt/axon/libaxon_pjrt.so"
        if not os.path.exists(so_path):
            return
        lib = ctypes.CDLL(so_path)
        if not hasattr(lib, "axon_start_nrt_profile"):
            return
        lib.axon_start_nrt_profile.argtypes = [
            ctypes.POINTER(ctypes.c_int64), ctypes.c_size_t]
        lib.axon_start_nrt_profile.restype = ctypes.c_int64
        lib.axon_stop_nrt_profile.argtypes = [ctypes.c_char_p]
        lib.axon_stop_nrt_profile.restype = ctypes.c_int64

        @contextlib.contextmanager
        def _hook(output_dir, device_ids):
            import jax

            jax.devices()
            if device_ids:
                ids = (ctypes.c_int64 * len(device_ids))(*device_ids)
                rc = lib.axon_start_nrt_profile(ids, len(device_ids))
            else:
                rc = lib.axon_start_nrt_profile(None, 0)
            if rc != 0:
                raise RuntimeError(f"axon_start_nrt_profile rc={rc}")
            try:
                yield
            finally:
                n = lib.axon_stop_nrt_profile(str(output_dir).encode())
                print(f"ntff profile: {n} file(s) -> {output_dir}")

        m = types.ModuleType("antenv.axon_hooks")
        m.get_axon_ntff_profile_hook = lambda: _hook
        m.set_axon_ntff_profile_hook = lambda h: None
        sys.modules["antenv.axon_hooks"] = m
        antenv.axon_hooks = m
    except Exception:
        pass


def _token_blocks(C):
    blocks = [(i * T, T) for i in range(C // T)]
    if C % T:
        blocks.append((C - C % T, C % T))
    return blocks


def build_program(C: int) -> "bacc.Bacc":
    assert C % 128 == 0
    blocks = _token_blocks(C)
    nblocks = len(blocks)

    nc = bacc.Bacc("TRN2", target_bir_lowering=False, debug=False,
                   num_devices=NCORES)

    xT = nc.dram_tensor("xT", [128, KT * C], BF16, kind="ExternalInput").ap()
    epsT = nc.dram_tensor("epsT", [128, KT * C], F32, kind="ExternalInput").ap()
    wnames = ["wm1", "wv1", "wm2", "wv2", "we1", "we2"]
    wdram = {n: nc.dram_tensor(n, [128, KT * 1024], BF16,
                               kind="ExternalInput").ap() for n in wnames}
    outs = {n: nc.dram_tensor(n, [128, KT * C], F32,
                              kind="ExternalOutput").ap()
            for n in ["muT", "lvT", "hT", "recT"]}

    Exp = mybir.ActivationFunctionType.Exp
    Copy = mybir.ActivationFunctionType.Copy
    mult = mybir.AluOpType.mult
    max_ = mybir.AluOpType.max
    add = mybir.AluOpType.add

    with tile.TileContext(nc) as tc:
        with (
            tc.tile_pool(name="wpool", bufs=1) as wpool,
            tc.tile_pool(name="io2", bufs=2) as io2,
            tc.tile_pool(name="io", bufs=1) as io,
            tc.tile_pool(name="mid", bufs=1) as mid,
            # Separate PSUM pools so the (one-block-delayed) decoder's
            # slot-recycling waits never reference encoder matmul progress.
            tc.tile_pool(name="psum_e", bufs=5,
                         space=bass.MemorySpace.PSUM) as psum_e,
            tc.tile_pool(name="psum_d", bufs=3,
                         space=bass.MemorySpace.PSUM) as psum_d,
        ):
            # Block-0 inputs first so their DMA descriptors lead the queues,
            # then resident weights in usage order: the first matmuls wait
            # only on x.k0 + wm1.k0, eps comes when the sampling needs it,
            # and the expert weights (first needed after enc(1)) come last.
            xt_tiles = {}
            eps_tiles = {}

            def fetch_block(b, x_only=False, eng=None):
                if b >= nblocks:
                    return
                eng = eng or nc.sync
                off, w = blocks[b]
                if b not in xt_tiles:
                    x = io2.tile([128, KT * w], BF16, tag="x")
                    eng.dma_start(x[:], xT[:, off * KT : off * KT + KT * w])
                    xt_tiles[b] = x
                if not x_only and b not in eps_tiles:
                    e = io2.tile([128, KT * w], F32, tag="eps")
                    eng.dma_start(e[:], epsT[:, off * KT : off * KT + KT * w])
                    eps_tiles[b] = e

            wt = {}

            def fetch_weight(name, eng=None):
                eng = eng or nc.sync
                w = wpool.tile([128, KT * 1024], BF16, tag=f"w_{name}")
                eng.dma_start(w[:], wdram[name][:])
                wt[name] = w

            fetch_block(0, x_only=True)
            for n in ["wm1", "wv1", "wm2", "wv2"]:
                fetch_weight(n)
            fetch_block(0)
            fetch_block(1)
            for n in ["we1", "we2"]:
                fetch_weight(n)

            def layer(w, rhs, tw, out_cb, pool=None, first_after=None):
                """One 1024->1024 matmul layer on a [128, KT*tw] bf16 rhs.

                out_cb(mp, ps) consumes the [128, 2*tw] f32 psum of m-pair mp.
                Returns the last matmul instruction.  first_after: ordering
                hint — schedule this layer's first matmul after that inst.
                """
                pool = pool or psum_e
                mm = None
                for mp in range(4):
                    ps = pool.tile([128, 2 * tw], F32, tag="ps")
                    for half in range(2):
                        m = 2 * mp + half
                        for k in range(KT):
                            mm = nc.tensor.matmul(
                                ps[:, half * tw : (half + 1) * tw],
                                w[:, k * 1024 + m * 128 : k * 1024 + (m + 1) * 128],
                                rhs[:, k * tw : (k + 1) * tw],
                                start=(k == 0),
                                stop=(k == KT - 1),
                            )
                            if first_after is not None:
                                tile.add_dep_helper(
                                    mm.ins, first_after.ins, sync=False,
                                    reason="decoder pipelined behind next block")
                                first_after = None
                    out_cb(mp, ps)
                return mm

            def leaky_to(dst, tw):
                def cb(mp, ps):
                    # leaky(x) = max(x, 0.01x); DVE can read PSUM only
                    # once per op, so stage 0.01x in SBUF first.
                    lk = io2.tile([128, 2 * tw], F32, tag="lk")
                    nc.vector.tensor_scalar_mul(lk[:], ps[:], SLOPE)
                    nc.vector.tensor_tensor(
                        dst[:, 2 * mp * tw : (2 * mp + 2) * tw],
                        lk[:], ps[:], max_)
                return cb

            def enc_block(b):
                """Encoder + sampling for block b; returns the bf16 h tile."""
                off, tw = blocks[b]
                x, epst = xt_tiles.pop(b), eps_tiles.pop(b)
                fetch_block(b + 1)

                h1m = mid.tile([128, KT * tw], BF16, tag="h1m")
                l1m_last = layer(wt["wm1"], x, tw, leaky_to(h1m, tw))
                h1v = mid.tile([128, KT * tw], BF16, tag="h1v")
                layer(wt["wv1"], x, tw, leaky_to(h1v, tw))

                mu_f = io.tile([128, KT * tw], F32, tag="mu_f")

                def mu_cb(mp, ps):
                    nc.scalar.activation(
                        mu_f[:, 2 * mp * tw : (2 * mp + 2) * tw], ps[:], Copy)

                layer(wt["wm2"], h1m, tw, mu_cb)
                nc.sync.dma_start(outs["muT"][:, off * KT : off * KT + KT * tw],
                                  mu_f[:])

                lv_f = io.tile([128, KT * tw], F32, tag="lv_f")
                std_f = mid.tile([128, KT * tw], F32, tag="std_f")
                tmp_f = mid.tile([128, KT * tw], F32, tag="tmp_f")
                h_f = io.tile([128, KT * tw], F32, tag="h_f")
                h_b = io2.tile([128, KT * tw], BF16, tag="h_b")

                def lv_cb(mp, ps):
                    sl = slice(2 * mp * tw, (2 * mp + 2) * tw)
                    nc.scalar.activation(lv_f[:, sl], ps[:], Copy)
                    nc.scalar.activation(std_f[:, sl], ps[:], Exp, scale=0.5)
                    # h = eps*std + mu, per m-pair so it pipelines
                    nc.vector.tensor_tensor(
                        tmp_f[:, sl], epst[:, sl], std_f[:, sl], mult)
                    nc.vector.tensor_tensor(
                        h_f[:, sl], tmp_f[:, sl], mu_f[:, sl], add)
                    nc.vector.tensor_tensor(
                        h_b[:, sl], tmp_f[:, sl], mu_f[:, sl], add)

                layer(wt["wv2"], h1v, tw, lv_cb)
                nc.sync.dma_start(outs["lvT"][:, off * KT : off * KT + KT * tw],
                                  lv_f[:])
                nc.sync.dma_start(outs["hT"][:, off * KT : off * KT + KT * tw],
                                  h_f[:])
                return h_b, l1m_last

            def dec_block(b, h_b, after=None):
                """Decoder (this core's expert) for block b."""
                off, tw = blocks[b]
                d1 = mid.tile([128, KT * tw], BF16, tag="d1")
                layer(wt["we1"], h_b, tw, leaky_to(d1, tw), pool=psum_d,
                      first_after=after)

                rec_f = io.tile([128, KT * tw], F32, tag="rec_f")

                def rec_cb(mp, ps):
                    sl = slice(2 * mp * tw, (2 * mp + 2) * tw)
                    nc.scalar.activation(rec_f[:, sl], ps[:], Copy)
                    # per-m-pair output DMA so the tail drains early
                    nc.sync.dma_start(
                        outs["recT"][:, off * KT + 2 * mp * tw :
                                     off * KT + (2 * mp + 2) * tw],
                        rec_f[:, sl])

                layer(wt["we2"], d1, tw, rec_cb, pool=psum_d)

            # Software-pipeline the decoder one block behind the encoder:
            # while block b's sampling chain (ACT exp + DVE fma) drains,
            # the PE is busy on block b-1's decoder — no PE idle at block
            # boundaries (which would also re-throttle the HAM clock).
            prev = None
            for b in range(nblocks):
                h_b, l1m_last = enc_block(b)
                if prev is not None:
                    dec_block(b - 1, prev, after=l1m_last)
                prev = h_b
            dec_block(nblocks - 1, prev)

    nc.compile()
    return nc


def _get_program(C: int) -> "bacc.Bacc":
    if C not in _program_cache:
        _program_cache[C] = build_program(C)
    return _program_cache[C]


def _to_sbuf_image(arrT, blocks):
    """[1024, C] feature-major -> [128, KT*C] flat SBUF image, blockwise."""
    out = np.empty((128, KT * arrT.shape[1]), dtype=arrT.dtype)
    for off, w in blocks:
        seg = arrT[:, off:off + w].reshape(KT, 128, w).transpose(1, 0, 2)
        out[:, off * KT : off * KT + KT * w] = seg.reshape(128, KT * w)
    return out


def _from_sbuf_image(img, blocks, C):
    """[128, KT*C] flat SBUF image -> [1024, C] feature-major."""
    out = np.empty((1024, C), dtype=img.dtype)
    for off, w in blocks:
        seg = img[:, off * KT : off * KT + KT * w].reshape(128, KT, w)
        out[:, off:off + w] = seg.transpose(1, 0, 2).reshape(1024, w)
    return out


def _weight_image(W):
    """[1024 din, 1024 dout] -> [128, KT*1024] flat lhsT image."""
    return np.ascontiguousarray(
        W.reshape(KT, 128, 1024).transpose(1, 0, 2).reshape(128, KT * 1024))


def _kernel_numpy(inputs):
    """Exact f32 fallback (used only if an assumption is violated)."""
    d = {k: np.asarray(v) for k, v in inputs.items()}
    leaky = lambda v: np.where(v > 0, v, np.float32(SLOPE) * v)
    mu = leaky(d["data"] @ d["Wm1"] + d["bm1"]) @ d["Wm2"] + d["bm2"]
    lv = leaky(d["data"] @ d["Wv1"] + d["bv1"]) @ d["Wv2"] + d["bv2"]
    h = d["eps"] * np.exp(0.5 * lv) + mu
    s = np.asarray(d["s"]).astype(np.int64)
    rec = np.empty_like(d["data"])
    for e in range(d["We1"].shape[0]):
        m = s == e
        rec[m] = leaky(h[m] @ d["We1"][e] + d["be1"][e]) @ d["We2"][e] + d["be2"][e]
    return rec, mu, lv, h


def kernel(**inputs) -> tuple:
    data = np.ascontiguousarray(np.asarray(inputs["data"], dtype=np.float32))
    eps = np.ascontiguousarray(np.asarray(inputs["eps"], dtype=np.float32))
    s = np.asarray(inputs["s"]).astype(np.int64)
    # The device kernel folds the (structurally zero) biases away; any
    # violated assumption falls back to an exact host computation.
    nonzero_bias = any(
        np.abs(np.asarray(inputs[b])).max() != 0.0
        for b in ("bm1", "bm2", "bv1", "bv2", "be1", "be2"))
    if nonzero_bias or data.shape != (N, DX) or s.shape != (N,):
        return _kernel_numpy(inputs)

    counts = np.bincount(s, minlength=S)
    C = max(C_MIN, int(math.ceil(counts.max() / 128)) * 128)
    blocks = _token_blocks(C)
    nc = _get_program(C)

    bf = ml_dtypes.bfloat16
    # token ids per expert, padded to C with token 0 (results discarded)
    idx = np.zeros((S, C), dtype=np.int64)
    for e in range(S):
        ids = np.nonzero(s == e)[0]
        idx[e, : len(ids)] = ids

    wimg = {
        "wm1": _weight_image(np.asarray(inputs["Wm1"], np.float32)).astype(bf),
        "wm2": _weight_image(np.asarray(inputs["Wm2"], np.float32)).astype(bf),
        "wv1": _weight_image(np.asarray(inputs["Wv1"], np.float32)).astype(bf),
        "wv2": _weight_image(np.asarray(inputs["Wv2"], np.float32)).astype(bf),
    }
    We1 = np.asarray(inputs["We1"], np.float32)
    We2 = np.asarray(inputs["We2"], np.float32)
    dataT = data.T
    epsT = eps.T

    in_maps = []
    for e in range(S):
        ids = idx[e]
        in_maps.append({
            "xT": _to_sbuf_image(
                np.ascontiguousarray(dataT[:, ids]).astype(bf), blocks),
            "epsT": _to_sbuf_image(
                np.ascontiguousarray(epsT[:, ids]), blocks),
            "wm1": wimg["wm1"], "wm2": wimg["wm2"],
            "wv1": wimg["wv1"], "wv2": wimg["wv2"],
            "we1": _weight_image(We1[e]).astype(bf),
            "we2": _weight_image(We2[e]).astype(bf),
        })

    global LAST_RESULTS
    _ensure_ntff_hook()
    res = run_bass_kernel_spmd(nc, in_maps, list(range(NCORES)))
    LAST_RESULTS = res

    mu = np.empty((N, DH), np.float32)
    lv = np.empty((N, DH), np.float32)
    h = np.empty((N, DH), np.float32)
    rec = np.empty((N, DX), np.float32)
    for e in range(S):
        cnt = int(counts[e])
        ids = idx[e, :cnt]
        r = res.results[e]
        mu[ids] = _from_sbuf_image(r["muT"], blocks, C)[:, :cnt].T
        lv[ids] = _from_sbuf_image(r["lvT"], blocks, C)[:, :cnt].T
        h[ids] = _from_sbuf_image(r["hT"], blocks, C)[:, :cnt].T
        rec[ids] = _from_sbuf_image(r["recT"], blocks, C)[:, :cnt].T
    return rec, mu, lv, h


# revision 41
# speedup vs baseline: 1.0526x; 1.0064x over previous
"""Trainium2 Bass kernel for nn_Causal_model_vae (MoE-routed VAE).

Reference computation (N=16384 tokens, DX=DH=1024, S=8 experts):
    mu_h     = leaky(data @ Wm1 + bm1) @ Wm2 + bm2
    logvar_h = leaky(data @ Wv1 + bv1) @ Wv2 + bv2
    h_sample = eps * exp(0.5*logvar_h) + mu_h
    reconstruct[n] = (leaky(h_sample @ We1[s_n] + be1[s_n]) @ We2[s_n] + be2[s_n])
returns (reconstruct, mu_h, logvar_h, h_sample).

Strategy: expert-parallel across the 8 NeuronCores.  The routing ids `s` are
known on the host, so the host sorts tokens by expert, pads each expert's
token list to a common capacity C (= max expert count rounded up to 128),
and core e processes exactly expert e's tokens: the (replicated-weight)
encoder on its C tokens, then ONLY its own expert's decoder — 6 matmul
layers per token instead of the reference's dense 4 + 2*S.

On-chip layout: everything is feature-major [feature, token] so chained
matmuls need no transposes; matmul operands are bf16 (f32 PSUM accumulation),
the sampling chain and all outputs are f32.  All DRAM tensors use the flat
SBUF-image layout [128, KT*width] (host pre/post-arranges) so every DMA is
fully contiguous per partition — strided DMAs cost ~2-3.5us of descriptor
generation on the sync sequencer and stall the pipeline.

The decoder is software-pipelined one token-block behind the encoder so the
PE never idles while the sampling chain (ACT exp + DVE fma) drains.

Biases are structurally zero in this problem's setup_inputs(); the kernel
asserts that and skips them on-device.
"""

import contextlib
import ctypes
import math
import os
import sys
import types

import numpy as np
import ml_dtypes

import concourse.bacc as bacc
import concourse.bass as bass
import concourse.mybir as mybir
import concourse.tile as tile
from concourse.bass_utils import run_bass_kernel_spmd

N, DX, DH, S = 16384, 1024, 1024, 8
KT = DH // 128    # 8 k-tiles (DX == DH == 1024)
SLOPE = 0.01
NCORES = 8
T = 256           # main token block width (matmul moving dim)
C_MIN = 256       # capacity floor; C = ceil(max expert count / 128) * 128

BF16 = mybir.dt.bfloat16
F32 = mybir.dt.float32

LAST_RESULTS = None  # BassKernelResults of the most recent run (for profiling)

_program_cache: dict[int, "bacc.Bacc"] = {}


def _ensure_ntff_hook():
    """bass_utils imports antenv.axon_hooks when tracing under axon; some
    images lack that module.  Install a ctypes-based equivalent if so."""
    try:
        import antenv.axon_hooks  # noqa: F401
        return
    except ImportError:
        pass
    try:
        import antenv

        so_path = "/opt/axon/libaxon_pjrt.so"
        if not os.path.exists(so_path):
            return
        lib = ctypes.CDLL(so_path)
        if not hasattr(lib, "axon_start_nrt_profile"):
            return
        lib.axon_start_nrt_profile.argtypes = [
            ctypes.POINTER(ctypes.c_int64), ctypes.c_size_t]
        lib.axon_start_nrt_profile.restype = ctypes.c_int64
        lib.axon_stop_nrt_profile.argtypes = [ctypes.c_char_p]
        lib.axon_stop_nrt_profile.restype = ctypes.c_int64

        @contextlib.contextmanager
        def _hook(output_dir, device_ids):
            import jax

            jax.devices()
            if device_ids:
                ids = (ctypes.c_int64 * len(device_ids))(*device_ids)
                rc = lib.axon_start_nrt_profile(ids, len(device_ids))
            else:
                rc = lib.axon_start_nrt_profile(None, 0)
            if rc != 0:
                raise RuntimeError(f"axon_start_nrt_profile rc={rc}")
            try:
                yield
            finally:
                n = lib.axon_stop_nrt_profile(str(output_dir).encode())
                print(f"ntff profile: {n} file(s) -> {output_dir}")

        m = types.ModuleType("antenv.axon_hooks")
        m.get_axon_ntff_profile_hook = lambda: _hook
        m.set_axon_ntff_profile_hook = lambda h: None
        sys.modules["antenv.axon_hooks"] = m
        antenv.axon_hooks = m
    except Exception:
        pass


def _token_blocks(C):
    blocks = [(i * T, T) for i in range(C // T)]
    if C % T:
        blocks.append((C - C % T, C % T))
    return blocks


def build_program(C: int) -> "bacc.Bacc":
    assert C % 128 == 0
    blocks = _token_blocks(C)
    nblocks = len(blocks)

    nc = bacc.Bacc("TRN2", target_bir_lowering=False, debug=False,
                   num_devices=NCORES)

    xT = nc.dram_tensor("xT", [128, KT * C], BF16, kind="ExternalInput").ap()
    epsT = nc.dram_tensor("epsT", [128, KT * C], F32, kind="ExternalInput").ap()
    wnames = ["wm1", "wv1", "wm2", "wv2", "we1", "we2"]
    wdram = {n: nc.dram_tensor(n, [128, KT * 1024], BF16,
                               kind="ExternalInput").ap() for n in wnames}
    outs = {n: nc.dram_tensor(n, [128, KT * C], F32,
                              kind="ExternalOutput").ap()
            for n in ["muT", "lvT", "hT", "recT"]}

    Exp = mybir.ActivationFunctionType.Exp
    Copy = mybir.ActivationFunctionType.Copy
    mult = mybir.AluOpType.mult
    max_ = mybir.AluOpType.max
    add = mybir.AluOpType.add

    with tile.TileContext(nc) as tc:
        with (
            tc.tile_pool(name="wpool", bufs=1) as wpool,
            tc.tile_pool(name="io2", bufs=2) as io2,
            tc.tile_pool(name="io", bufs=1) as io,
            tc.tile_pool(name="mid", bufs=1) as mid,
            # Separate PSUM pools so the (one-block-delayed) decoder's
            # slot-recycling waits never reference encoder matmul progress.
            tc.tile_pool(name="psum_e", bufs=5,
                         space=bass.MemorySpace.PSUM) as psum_e,
            tc.tile_pool(name="psum_d", bufs=3,
                         space=bass.MemorySpace.PSUM) as psum_d,
        ):
            # Block-0 inputs first so their DMA descriptors lead the queues,
            # then resident weights in usage order: the first matmuls wait
            # only on x.k0 + wm1.k0, eps comes when the sampling needs it,
            # and the expert weights (first needed after enc(1)) come last.
            xt_tiles = {}
            eps_tiles = {}

            def fetch_block(b, x_only=False, eng=None):
                if b >= nblocks:
                    return
                eng = eng or nc.sync
                off, w = blocks[b]
                if b not in xt_tiles:
                    x = io2.tile([128, KT * w], BF16, tag="x")
                    eng.dma_start(x[:], xT[:, off * KT : off * KT + KT * w])
                    xt_tiles[b] = x
                if not x_only and b not in eps_tiles:
                    e = io2.tile([128, KT * w], F32, tag="eps")
                    eng.dma_start(e[:], epsT[:, off * KT : off * KT + KT * w])
                    eps_tiles[b] = e

            wt = {}

            def fetch_weight(name, eng=None):
                eng = eng or nc.sync
                w = wpool.tile([128, KT * 1024], BF16, tag=f"w_{name}")
                eng.dma_start(w[:], wdram[name][:])
                wt[name] = w

            # PE warm-up: the first real matmul waits ~9us for x0+wm1 DMA
            # (2.5MB at queue bandwidth), during which HAM throttles the PE
            # to 1.2GHz.  Run dependency-free dummy matmuls on a zeroed tile
            # in that window so the real stream starts at 2.4GHz.
            warm = io2.tile([128, 256], BF16, tag="warm")
            nc.gpsimd.memset(warm[:], 0.0)
            ps_w = psum_e.tile([128, 256], F32, tag="ps")
            for _ in range(48):
                nc.tensor.matmul(ps_w[:], warm[:, :128], warm[:],
                                 start=True, stop=True)

            fetch_block(0, x_only=True)
            for n in ["wm1", "wv1", "wm2", "wv2"]:
                fetch_weight(n)
            fetch_block(0)
            fetch_block(1)
            for n in ["we1", "we2"]:
                fetch_weight(n)

            def layer(w, rhs, tw, out_cb, pool=None, first_after=None):
                """One 1024->1024 matmul layer on a [128, KT*tw] bf16 rhs.

                out_cb(mp, ps) consumes the [128, 2*tw] f32 psum of m-pair mp.
                Returns the last matmul instruction.  first_after: ordering
                hint — schedule this layer's first matmul after that inst.
                """
                pool = pool or psum_e
                mm = None
                for mp in range(4):
                    ps = pool.tile([128, 2 * tw], F32, tag="ps")
                    for half in range(2):
                        m = 2 * mp + half
                        for k in range(KT):
                            mm = nc.tensor.matmul(
                                ps[:, half * tw : (half + 1) * tw],
                                w[:, k * 1024 + m * 128 : k * 1024 + (m + 1) * 128],
                                rhs[:, k * tw : (k + 1) * tw],
                                start=(k == 0),
                                stop=(k == KT - 1),
                            )
                            if first_after is not None:
                                tile.add_dep_helper(
                                    mm.ins, first_after.ins, sync=False,
                                    reason="decoder pipelined behind next block")
                                first_after = None
                    out_cb(mp, ps)
                return mm

            def leaky_to(dst, tw):
                def cb(mp, ps):
                    # leaky(x) = max(x, 0.01x); DVE can read PSUM only
                    # once per op, so stage 0.01x in SBUF first.
                    lk = io2.tile([128, 2 * tw], F32, tag="lk")
                    nc.vector.tensor_scalar_mul(lk[:], ps[:], SLOPE)
                    nc.vector.tensor_tensor(
                        dst[:, 2 * mp * tw : (2 * mp + 2) * tw],
                        lk[:], ps[:], max_)
                return cb

            def enc_block(b):
                """Encoder + sampling for block b; returns the bf16 h tile."""
                off, tw = blocks[b]
                x, epst = xt_tiles.pop(b), eps_tiles.pop(b)
                fetch_block(b + 1)

                h1m = mid.tile([128, KT * tw], BF16, tag="h1m")
                l1m_last = layer(wt["wm1"], x, tw, leaky_to(h1m, tw))
                h1v = mid.tile([128, KT * tw], BF16, tag="h1v")
                layer(wt["wv1"], x, tw, leaky_to(h1v, tw))

                mu_f = io.tile([128, KT * tw], F32, tag="mu_f")

                def mu_cb(mp, ps):
                    nc.scalar.activation(
                        mu_f[:, 2 * mp * tw : (2 * mp + 2) * tw], ps[:], Copy)

                layer(wt["wm2"], h1m, tw, mu_cb)
                nc.sync.dma_start(outs["muT"][:, off * KT : off * KT + KT * tw],
                                  mu_f[:])

                lv_f = io.tile([128, KT * tw], F32, tag="lv_f")
                std_f = mid.tile([128, KT * tw], F32, tag="std_f")
                tmp_f = mid.tile([128, KT * tw], F32, tag="tmp_f")
                h_f = io.tile([128, KT * tw], F32, tag="h_f")
                h_b = io2.tile([128, KT * tw], BF16, tag="h_b")

                def lv_cb(mp, ps):
                    sl = slice(2 * mp * tw, (2 * mp + 2) * tw)
                    nc.scalar.activation(lv_f[:, sl], ps[:], Copy)
                    nc.scalar.activation(std_f[:, sl], ps[:], Exp, scale=0.5)
                    # h = eps*std + mu, per m-pair so it pipelines
                    nc.vector.tensor_tensor(
                        tmp_f[:, sl], epst[:, sl], std_f[:, sl], mult)
                    nc.vector.tensor_tensor(
                        h_f[:, sl], tmp_f[:, sl], mu_f[:, sl], add)
                    nc.vector.tensor_tensor(
                        h_b[:, sl], tmp_f[:, sl], mu_f[:, sl], add)

                layer(wt["wv2"], h1v, tw, lv_cb)
                nc.sync.dma_start(outs["lvT"][:, off * KT : off * KT + KT * tw],
                                  lv_f[:])
                nc.sync.dma_start(outs["hT"][:, off * KT : off * KT + KT * tw],
                                  h_f[:])
                return h_b, l1m_last

            def dec_block(b, h_b, after=None):
                """Decoder (this core's expert) for block b."""
                off, tw = blocks[b]
                d1 = mid.tile([128, KT * tw], BF16, tag="d1")
                layer(wt["we1"], h_b, tw, leaky_to(d1, tw), pool=psum_d,
                      first_after=after)

                rec_f = io.tile([128, KT * tw], F32, tag="rec_f")

                def rec_cb(mp, ps):
                    sl = slice(2 * mp * tw, (2 * mp + 2) * tw)
                    nc.scalar.activation(rec_f[:, sl], ps[:], Copy)
                    # per-m-pair output DMA so the tail drains early
                    nc.sync.dma_start(
                        outs["recT"][:, off * KT + 2 * mp * tw :
                                     off * KT + (2 * mp + 2) * tw],
                        rec_f[:, sl])

                layer(wt["we2"], d1, tw, rec_cb, pool=psum_d)

            # Software-pipeline the decoder one block behind the encoder:
            # while block b's sampling chain (ACT exp + DVE fma) drains,
            # the PE is busy on block b-1's decoder — no PE idle at block
            # boundaries (which would also re-throttle the HAM clock).
            prev = None
            for b in range(nblocks):
                h_b, l1m_last = enc_block(b)
                if prev is not None:
                    dec_block(b - 1, prev, after=l1m_last)
                prev = h_b
            dec_block(nblocks - 1, prev)

    nc.compile()
    return nc


def _get_program(C: int) -> "bacc.Bacc":
    if C not in _program_cache:
        _program_cache[C] = build_program(C)
    return _program_cache[C]


def _to_sbuf_image(arrT, blocks):
    """[1024, C] feature-major -> [128, KT*C] flat SBUF image, blockwise."""
    out = np.empty((128, KT * arrT.shape[1]), dtype=arrT.dtype)
    for off, w in blocks:
        seg = arrT[:, off:off + w].reshape(KT, 128, w).transpose(1, 0, 2)
        out[:, off * KT : off * KT + KT * w] = seg.reshape(128, KT * w)
    return out


def _from_sbuf_image(img, blocks, C):
    """[128, KT*C] flat SBUF image -> [1024, C] feature-major."""
    out = np.empty((1024, C), dtype=img.dtype)
    for off, w in blocks:
        seg = img[:, off * KT : off * KT + KT * w].reshape(128, KT, w)
        out[:, off:off + w] = seg.transpose(1, 0, 2).reshape(1024, w)
    return out


def _weight_image(W):
    """[1024 din, 1024 dout] -> [128, KT*1024] flat lhsT image."""
    return np.ascontiguousarray(
        W.reshape(KT, 128, 1024).transpose(1, 0, 2).reshape(128, KT * 1024))


def _kernel_numpy(inputs):
    """Exact f32 fallback (used only if an assumption is violated)."""
    d = {k: np.asarray(v) for k, v in inputs.items()}
    leaky = lambda v: np.where(v > 0, v, np.float32(SLOPE) * v)
    mu = leaky(d["data"] @ d["Wm1"] + d["bm1"]) @ d["Wm2"] + d["bm2"]
    lv = leaky(d["data"] @ d["Wv1"] + d["bv1"]) @ d["Wv2"] + d["bv2"]
    h = d["eps"] * np.exp(0.5 * lv) + mu
    s = np.asarray(d["s"]).astype(np.int64)
    rec = np.empty_like(d["data"])
    for e in range(d["We1"].shape[0]):
        m = s == e
        rec[m] = leaky(h[m] @ d["We1"][e] + d["be1"][e]) @ d["We2"][e] + d["be2"][e]
    return rec, mu, lv, h


def kernel(**inputs) -> tuple:
    data = np.ascontiguousarray(np.asarray(inputs["data"], dtype=np.float32))
    eps = np.ascontiguousarray(np.asarray(inputs["eps"], dtype=np.float32))
    s = np.asarray(inputs["s"]).astype(np.int64)
    # The device kernel folds the (structurally zero) biases away; any
    # violated assumption falls back to an exact host computation.
    nonzero_bias = any(
        np.abs(np.asarray(inputs[b])).max() != 0.0
        for b in ("bm1", "bm2", "bv1", "bv2", "be1", "be2"))
    if nonzero_bias or data.shape != (N, DX) or s.shape != (N,):
        return _kernel_numpy(inputs)

    counts = np.bincount(s, minlength=S)
    C = max(C_MIN, int(math.ceil(counts.max() / 128)) * 128)
    blocks = _token_blocks(C)
    nc = _get_program(C)

    bf = ml_dtypes.bfloat16
    # token ids per expert, padded to C with token 0 (results discarded)
    idx = np.zeros((S, C), dtype=np.int64)
    for e in range(S):
        ids = np.nonzero(s == e)[0]
        idx[e, : len(ids)] = ids

    wimg = {
        "wm1": _weight_image(np.asarray(inputs["Wm1"], np.float32)).astype(bf),
        "wm2": _weight_image(np.asarray(inputs["Wm2"], np.float32)).astype(bf),
        "wv1": _weight_image(np.asarray(inputs["Wv1"], np.float32)).astype(bf),
        "wv2": _weight_image(np.asarray(inputs["Wv2"], np.float32)).astype(bf),
    }
    We1 = np.asarray(inputs["We1"], np.float32)
    We2 = np.asarray(inputs["We2"], np.float32)
    dataT = data.T
    epsT = eps.T

    in_maps = []
    for e in range(S):
        ids = idx[e]
        in_maps.append({
            "xT": _to_sbuf_image(
                np.ascontiguousarray(dataT[:, ids]).astype(bf), blocks),
            "epsT": _to_sbuf_image(
                np.ascontiguousarray(epsT[:, ids]), blocks),
            "wm1": wimg["wm1"], "wm2": wimg["wm2"],
            "wv1": wimg["wv1"], "wv2": wimg["wv2"],
            "we1": _weight_image(We1[e]).astype(bf),
            "we2": _weight_image(We2[e]).astype(bf),
        })

    global LAST_RESULTS
    _ensure_ntff_hook()
    res = run_bass_kernel_spmd(nc, in_maps, list(range(NCORES)))
    LAST_RESULTS = res

    mu = np.empty((N, DH), np.float32)
    lv = np.empty((N, DH), np.float32)
    h = np.empty((N, DH), np.float32)
    rec = np.empty((N, DX), np.float32)
    for e in range(S):
        cnt = int(counts[e])
        ids = idx[e, :cnt]
        r = res.results[e]
        mu[ids] = _from_sbuf_image(r["muT"], blocks, C)[:, :cnt].T
        lv[ids] = _from_sbuf_image(r["lvT"], blocks, C)[:, :cnt].T
        h[ids] = _from_sbuf_image(r["hT"], blocks, C)[:, :cnt].T
        rec[ids] = _from_sbuf_image(r["recT"], blocks, C)[:, :cnt].T
    return rec, mu, lv, h
